# revision 2
# baseline (speedup 1.0000x reference)
"""MoE gate (nn_MoEGate) Trainium2 Bass kernel — self-contained.

Problem: x (8192, 8, 1024) f32, weight (31, 1024) f32.
reference: logits = x.reshape(-1,1024) @ weight.T; softmax; top-3 (idx, renormalized
weights); seq-aux loss over 8 groups of 8192 consecutive tokens.

Sharding: data-parallel over the flattened token dim. 65536 tokens are split into
8 contiguous shards of 8192, one per NeuronCore; the (31,1024) gate weight is
replicated. Each shard coincides exactly with one "batch" group of the seq-aux
loss, so the aux loss needs no cross-core reduction: each core emits per-expert
selection counts and softmax-prob sums, and the host combines 8 scalars.

Per-core pipeline (fp32 end to end on the PE):
  - DMA 512-token x slabs HBM->SBUF (natural token-major layout)
  - PE transpose-mode 128x128 tiles to put d on partitions (xT), via PSUM
  - DVE/ACT copy PSUM->SBUF
  - PE matmul wT.T @ xT -> logitsT (31, 512) PSUM, accumulated over 8 d-chunks
  - PE transpose logits back to token-major (128, 31)
  - ACT exp + DVE softmax/top-k epilogue:
      max8+max_index8 give the top-3 experts per token (exact fp32 compare);
      top-3 weights = softmax over the 3 top logits (the full-softmax
      normalizer cancels); counts via (logit >= 3rd-max) mask.
"""
import threading

import numpy as np

S, B, D = 8192, 8, 1024
E, K = 31, 3
N_CORES = 8
T_CORE = (S * B) // N_CORES  # 8192 tokens per core
ALPHA = 0.001

F32 = None  # set on first build (lazy concourse import)

_lock = threading.Lock()
_cache = {}


def _build():
    from contextlib import ExitStack

    import concourse.mybir as mybir
    from concourse import bacc
    from concourse.bass import ds
    from concourse.tile import TileContext
    from concourse.masks import make_identity
    from concourse.alu_op_type import AluOpType

    F32 = mybir.dt.float32
    I32 = mybir.dt.int32
    U32 = mybir.dt.uint32
    AF = mybir.ActivationFunctionType
    AX = mybir.AxisListType

    T = T_CORE
    NCHUNK = D // 128       # 8
    NG = T // 512           # 16 groups of 512 tokens
    NBLK = T // 128         # 64 blocks of 128 tokens
    GEP = 8                 # epilogue batch: 8 blocks = 1024 tokens
    GPE = GEP // 4          # 512-groups per epilogue batch
    MM_ORIENT = 2
    COPY_SPLIT = 320        # PSUM->SBUF xT copy: [0:split] on DVE, rest on ACT

    nc = bacc.Bacc("TRN2", target_bir_lowering=False, debug=False,
                   num_devices=N_CORES)

    x_dram = nc.dram_tensor("x", (T, D), F32, kind="ExternalInput").ap()
    w_dram = nc.dram_tensor("w", (E, D), F32, kind="ExternalInput").ap()
    wout_dram = nc.dram_tensor("wout", (128, NBLK, 3), F32, kind="ExternalOutput").ap()
    iout_dram = nc.dram_tensor("iout", (128, NBLK, 3), I32, kind="ExternalOutput").ap()
    cnt_dram = nc.dram_tensor("cnt", (128, E), F32, kind="ExternalOutput").ap()
    psm_dram = nc.dram_tensor("psm", (128, E), F32, kind="ExternalOutput").ap()

    xg = x_dram.rearrange("(g s p) d -> g s p d", s=4, p=128)

    with TileContext(nc) as tc, ExitStack() as ctx:
        const = ctx.enter_context(tc.tile_pool(name="const", bufs=1))
        xin = ctx.enter_context(tc.tile_pool(name="xin", bufs=3))
        xtp = ctx.enter_context(tc.tile_pool(name="xtp", bufs=3))
        lts = ctx.enter_context(tc.tile_pool(name="lts", bufs=2))
        epi = ctx.enter_context(tc.tile_pool(name="epi", bufs=2))
        acc = ctx.enter_context(tc.tile_pool(name="acc", bufs=1))
        outp = ctx.enter_context(tc.tile_pool(name="outp", bufs=1))
        ps_x = ctx.enter_context(tc.tile_pool(name="ps_x", bufs=3, space="PSUM"))
        ps_l = ctx.enter_context(tc.tile_pool(name="ps_l", bufs=2, space="PSUM"))
        ps_t = ctx.enter_context(tc.tile_pool(name="ps_t", bufs=2, space="PSUM"))

        ident = const.tile([128, 128], F32)
        make_identity(nc, ident)

        w_sb = const.tile([E, D], F32)
        nc.sync.dma_start(w_sb[:], w_dram)
        wT = const.tile([128, NCHUNK, E], F32)
        for c in range(NCHUNK):
            pw = ps_t.tile([128, E], F32, tag="pw", bufs=1)
            nc.tensor.transpose(pw[:], w_sb[:, ds(c * 128, 128)], ident[:E, :E])
            nc.vector.tensor_copy(wT[:, c, :], pw[:])

        accC = acc.tile([128, E], F32)
        accP = acc.tile([128, E], F32)
        nc.vector.memset(accC[:], 0.0)
        nc.vector.memset(accP[:], 0.0)

        w_out = outp.tile([128, NBLK, 3], F32)
        i_out = outp.tile([128, NBLK, 3], I32)

        for b in range(NG // GPE):
            p_sb = epi.tile([128, GEP, E], F32, tag="p")
            l_sb = epi.tile([128, GEP, E], F32, tag="l")
            mx = epi.tile([128, GEP, 8], F32, tag="mx")
            mi = epi.tile([128, GEP, 8], U32, tag="mi")

            for gg in range(GPE):
                g = b * GPE + gg
                x_sb = xin.tile([128, 4, D], F32, tag="x")
                nc.sync.dma_start(x_sb[:], xg[g].rearrange("s p d -> p s d"))

                if MM_ORIENT == 2:
                    lg_ps = ps_l.tile([E, 512], F32, tag="lg")
                else:
                    lg_ps = ps_l.tile([128, 4, E], F32, tag="lg")

                for c in range(NCHUNK):
                    px = ps_x.tile([128, 512], F32, tag="px")
                    for s in range(4):
                        nc.tensor.transpose(
                            px[:, ds(s * 128, 128)],
                            x_sb[:, s, ds(c * 128, 128)],
                            ident[:],
                        )
                    xT = xtp.tile([128, 512], F32, tag="xT")
                    if COPY_SPLIT > 0:
                        nc.vector.tensor_copy(xT[:, :COPY_SPLIT], px[:, :COPY_SPLIT])
                    if COPY_SPLIT < 512:
                        nc.scalar.copy(xT[:, COPY_SPLIT:], px[:, COPY_SPLIT:])
                    if MM_ORIENT == 2:
                        nc.tensor.matmul(
                            lg_ps[:], wT[:, c, :], xT[:],
                            start=(c == 0), stop=(c == NCHUNK - 1),
                        )
                    else:
                        for s in range(4):
                            nc.tensor.matmul(
                                lg_ps[:, s, :], xT[:, ds(s * 128, 128)], wT[:, c, :],
                                start=(c == 0), stop=(c == NCHUNK - 1),
                            )

                if MM_ORIENT == 2:
                    lT_sb = lts.tile([E, 512], F32, tag="lT")
                    nc.vector.tensor_copy(lT_sb[:], lg_ps[:])
                    pl = ps_t.tile([128, 4, E], F32, tag="pl")
                    for s in range(4):
                        nc.tensor.transpose(
                            pl[:, s, :], lT_sb[:, ds(s * 128, 128)], ident[:E, :E]
                        )
                    lsrc = pl
                else:
                    lsrc = lg_ps

                for s in range(4):
                    gs = gg * 4 + s
                    nc.scalar.activation(p_sb[:, gs, :], lsrc[:, s, :], AF.Exp)
                    nc.vector.tensor_copy(l_sb[:, gs, :], lsrc[:, s, :])
                    nc.vector.max(out=mx[:, gs, :], in_=l_sb[:, gs, :])
                    nc.vector.max_index(
                        out=mi[:, gs, :], in_max=mx[:, gs, :], in_values=l_sb[:, gs, :]
                    )

            Ssb = epi.tile([128, GEP], F32, tag="S")
            nc.vector.tensor_reduce(Ssb[:], p_sb[:], AX.X, AluOpType.add)
            nc.vector.reciprocal(Ssb[:], Ssb[:])
            nc.vector.tensor_tensor(
                p_sb[:], p_sb[:],
                Ssb[:, :, None].to_broadcast([128, GEP, E]), AluOpType.mult,
            )
            tmpP = epi.tile([128, E], F32, tag="tmpP")
            nc.vector.tensor_reduce(
                tmpP[:], p_sb[:].rearrange("p g e -> p e g"), AX.X, AluOpType.add
            )
            nc.vector.tensor_add(accP[:], accP[:], tmpP[:])

            msk = epi.tile([128, GEP, E], F32, tag="msk")
            nc.vector.tensor_tensor(
                msk[:], l_sb[:],
                mx[:, :, 2][:, :, None].to_broadcast([128, GEP, E]), AluOpType.is_ge,
            )
            tmpC = epi.tile([128, E], F32, tag="tmpC")
            nc.vector.tensor_reduce(
                tmpC[:], msk[:].rearrange("p g e -> p e g"), AX.X, AluOpType.add
            )
            nc.vector.tensor_add(accC[:], accC[:], tmpC[:])

            e3 = epi.tile([128, GEP, 3], F32, tag="e3")
            nc.scalar.activation(e3[:], mx[:, :, 0:3], AF.Exp)
            s3 = epi.tile([128, GEP], F32, tag="s3")
            nc.vector.tensor_reduce(s3[:], e3[:], AX.X, AluOpType.add)
            nc.vector.reciprocal(s3[:], s3[:])
            nc.vector.tensor_tensor(
                w_out[:, ds(b * GEP, GEP), :], e3[:],
                s3[:, :, None].to_broadcast([128, GEP, 3]), AluOpType.mult,
            )
            nc.vector.tensor_copy(i_out[:, ds(b * GEP, GEP), :], mi[:, :, 0:3])

        nc.sync.dma_start(wout_dram, w_out[:])
        nc.sync.dma_start(iout_dram, i_out[:])
        nc.sync.dma_start(cnt_dram, accC[:])
        nc.sync.dma_start(psm_dram, accP[:])

    nc.compile()
    return nc


def _make_runner():
    """Build the kernel once and return a callable(per_core_in_maps) -> list of
    per-core output dicts, backed by a persistent jax.jit over shard_map."""
    import jax
    import numpy as np
    from jax.sharding import Mesh, PartitionSpec
    from jax.experimental.shard_map import shard_map

    import concourse.mybir as mybir
    from concourse.bass2jax import (
        _bass_exec_p,
        install_neuronx_cc_hook,
        partition_id_tensor,
    )

    nc = _build()
    install_neuronx_cc_hook()

    partition_name = nc.partition_id_tensor.name if nc.partition_id_tensor else None

    in_names, out_names, out_avals, zero_outs = [], [], [], []
    for alloc in nc.m.functions[0].allocations:
        if not isinstance(alloc, mybir.MemoryLocationSet):
            continue
        name = alloc.memorylocations[0].name
        if alloc.kind == "ExternalInput":
            if name != partition_name:
                in_names.append(name)
        elif alloc.kind == "ExternalOutput":
            shape = tuple(alloc.tensor_shape)
            dtype = mybir.dt.np(alloc.dtype)
            out_names.append(name)
            out_avals.append(jax.core.ShapedArray(shape, dtype))
            zero_outs.append(np.zeros(shape, dtype))
    n_params = len(in_names)
    n_outs = len(out_avals)
    all_names = in_names + out_names
    if partition_name is not None:
        all_names = all_names + [partition_name]

    def _body(*args):
        operands = list(args)
        if partition_name is not None:
            operands.append(partition_id_tensor())
        outs = _bass_exec_p.bind(
            *operands,
            out_avals=tuple(out_avals),
            in_names=tuple(all_names),
            out_names=tuple(out_names),
            lowering_input_output_aliases=(),
            sim_require_finite=True,
            sim_require_nnan=True,
            nc=nc,
        )
        return tuple(outs)

    devices = jax.devices()[:N_CORES]
    mesh = Mesh(np.asarray(devices), ("core",))
    in_specs = (PartitionSpec("core"),) * (n_params + n_outs)
    out_specs = (PartitionSpec("core"),) * n_outs
    sharded = jax.jit(
        shard_map(_body, mesh=mesh, in_specs=in_specs, out_specs=out_specs,
                  check_rep=False),
        donate_argnums=tuple(range(n_params, n_params + n_outs)),
        keep_unused=True,
    )

    def run(per_core_in_maps, device_inputs=None):
        if device_inputs is None:
            concat_in = [
                np.concatenate([m[name] for m in per_core_in_maps], axis=0)
                for name in in_names
            ]
        else:
            concat_in = device_inputs
        concat_zeros = [
            np.zeros((N_CORES * z.shape[0], *z.shape[1:]), z.dtype)
            for z in zero_outs
        ]
        out_arrs = sharded(*concat_in, *concat_zeros)
        return out_arrs

    def to_results(out_arrs):
        return [
            {
                name: np.asarray(out_arrs[i]).reshape(
                    N_CORES, *out_avals[i].shape)[c]
                for i, name in enumerate(out_names)
            }
            for c in range(N_CORES)
        ]

    return run, to_results, in_names, mesh


def _get_runner():
    with _lock:
        if "runner" not in _cache:
            _cache["runner"] = _make_runner()
        return _cache["runner"]


def _assemble(results):
    """Combine per-core outputs into full (topk_idx, topk_weight, aux_loss)."""
    idx_parts, w_parts, s_parts = [], [], []
    for c in range(N_CORES):
        out = results[c]
        # token t_local = p + 128*g  ->  (T_CORE, 3)
        idx_parts.append(out["iout"].transpose(1, 0, 2).reshape(T_CORE, 3))
        w_parts.append(out["wout"].transpose(1, 0, 2).reshape(T_CORE, 3))
        cnt = out["cnt"].astype(np.float64).sum(axis=0)      # (31,)
        psm = out["psm"].astype(np.float64).sum(axis=0)      # (31,)
        ce = cnt / (S * K / E)
        mean_sc = psm / S
        s_parts.append(float((ce * mean_sc).sum()))
    topk_idx = np.ascontiguousarray(np.concatenate(idx_parts, axis=0))
    topk_w = np.ascontiguousarray(np.concatenate(w_parts, axis=0))
    aux = np.float32(np.mean(s_parts) * ALPHA)
    return topk_idx.astype(np.int32), topk_w.astype(np.float32), aux


def kernel(x: np.ndarray, weight: np.ndarray):
    """Full-input entry point: x (8192, 8, 1024) f32, weight (31, 1024) f32.

    Returns (topk_idx (65536,3) int32, topk_weight (65536,3) f32, aux_loss f32)
    matching reference().
    """
    x = np.ascontiguousarray(np.asarray(x, dtype=np.float32))
    weight = np.ascontiguousarray(np.asarray(weight, dtype=np.float32))
    xf = x.reshape(S * B, D)

    run, to_results, in_names, _mesh = _get_runner()
    per_core = [
        {"x": xf[c * T_CORE:(c + 1) * T_CORE], "w": weight}
        for c in range(N_CORES)
    ]
    out_arrs = run(per_core)
    results = to_results(out_arrs)
    return _assemble(results)
